# revision 72
# baseline (speedup 1.0000x reference)
"""AttentionBlock (GroupNorm -> qkv -> MHA -> proj -> residual) on 8 trn2 cores.

Data-parallel over batch: 16 batches -> 2 per core. No collectives.

Per-core math (per batch item, c=512 channels, hw=1024 spatial, 8 heads x 64):
  xn = groupnorm(x)                     [c, hw] layout, stored e4m3 DR pairs
  q,k = Wqk^T.T @ xn (fp8 DoubleRow)    [2c, hw] -> bf16 SBUF (S path)
  vT  = xn.T @ WvT (fp8 DR)             [hw, c] e4m3 (+ 1/16 ones col)
  per head: S^T = kpad^T q (bf16 K=128) [s, t]
            P = exp(S^T / 8)            e5m2 (ACT table exp / DVE Schraudolph)
            AV fp8 DR K=256             rows 0..64 unnorm out, row 64 = r/16
            h*16 = AV[0:64] * (16/r)    e4m3 hts
  y = x + (WprojT.T @ hts)(fp8 DR)/16

vs the 257us bf16 version (this file's ancestor), trace-driven changes
(257 -> ~204us measured; engine totals ACT ~123 / DVE ~135 / PE ~148us):
  - fp8 DoubleRow for ALL K>=256 matmuls (qkv, vt, proj): xn/weights/hts in
    e4m3, contraction pairs over kt tiles (dual-subtile middle dim on a
    [128, pair, 2, col] wpack).  Halves those phases' PE instructions.
    Errors are diluted ~16x because ||h_attn|| ~ 0.07*||x|| (softmax
    averaging): measured rel err 7.5e-3 vs 3.6e-3 for the bf16 ancestor.
  - pairs run as two sequential j-phases sharing ONE [128,1024] AV psum
    accumulator (2 banks), freeing 6 banks for THREE full-width S slots ->
    exps are single [128,1024] ops (vs [128,512] chunks: half the
    instruction + semaphore overhead).  Phase st-cadence is ~1.0-1.3us.
  - ALL AV matmuls emitted after the phase's S stream (an AV waiting on an
    exp head-of-line blocks the S matmuls behind it in the PE FIFO); j1's
    first two S matmuls emitted before the AV-j0 block; j0's drain copies
    emitted after j1's first exps (ACT FIFO would block them).
  - exp engine split per (j, st): ACT_TILES on table-exp ScalarE, rest on
    DVE via Schraudolph e5m2 bit-affine (9/7 split balances ACT vs DVE
    incl. their drain work).
  - x shipped bf16 (DMA fabric here moves ~50-80GB/s effective; f32 x was
    gating groupnorm stats at ~28us), two cts per dma_start (each start
    costs ~0.6us of Sync issue time); gm/em one-hots shipped bf16 in a
    separate cpb (f32 one-hots made the gn matmuls run 2-pass fp32 mode).
  - HAM warmup matmuls bridge t=0 -> xn-ready (~18us): without them the
    whole vt/qk phase of b=0 ran at 1.2GHz.  WARM0 before gn(0), WARM1
    after (tuned empirically; more is NOT better - they delay real work).
  - softmax denominators: r rows ACT-copied to SBUF (reciprocal_approx_fast
    returns garbage on PSUM inputs - hardware-verified), one batched recip,
    f32 DMA broadcast roundtrip, deferred GpSimd hts multiplies.  Last
    pair: j0 via the roundtrip (frees the single AV accumulator fast), j1
    via a K=1 bf16 ones-matmul broadcast + DVE multiply (fp32 matmuls here
    run 2-pass LOW/HIGH at ~1.1us per MM - use bf16).
  - gn stats(1) split across two pair boundaries; b1 stats/apply emitted
    mid-pair-region where DVE has slack.
  - memsets on GpSimd; zero-padded k for S (HAM clock gate; note the pad
    costs NO PE cycles - matmul time is N columns regardless of K) and the
    e5m2 Schraudolph exp kept from the ancestor.
"""

import os

import numpy as np
import ml_dtypes

import concourse.bass as bass
import concourse.tile as tile
import concourse.mybir as mybir
from concourse import bacc

NUM_HEADS = 8
NUM_GROUPS = 32
EPS = 1e-5
B, C, H, W = 16, 512, 32, 32
HW = H * W                  # 1024
NCORES = 8
BPC = B // NCORES           # 2 batches per core
HD = C // NUM_HEADS         # 64
GS = C // NUM_GROUPS        # 16 channels per group
CT = C // 128               # 4 channel tiles
NP = CT // 2                # 2 channel-tile pairs (fp8 DR contraction)
QKT = 2 * C // 128          # 8 q+k output tiles
ST = HW // 128              # 8 sequence tiles
NH = HW // 512              # 2 moving-dim chunks of 512

F32 = mybir.dt.float32
BF16 = mybir.dt.bfloat16
F8E4 = mybir.dt.float8e4
F8E5 = mybir.dt.float8e5
I16 = mybir.dt.int16
I8 = mybir.dt.int8
U32 = mybir.dt.uint32
ALU = mybir.AluOpType
ACTF = mybir.ActivationFunctionType
DR = mybir.MatmulPerfMode.DoubleRow

USE_GP = os.environ.get("KERNEL_NO_GP") != "1"      # hts/xn work on gpsimd
# r rows must bounce through SBUF: reciprocal_approx_fast (custom DVE op)
# returns garbage on PSUM inputs (hardware-verified: direct-PSUM -> NaN out)
SAFE_R = os.environ.get("KERNEL_SAFE_R", "1") == "1"
SAFE_LAST = os.environ.get("KERNEL_SAFE_LAST") == "1"  # last pair uses DMA roundtrip
USE_WARM = os.environ.get("KERNEL_NO_WARM") != "1"  # HAM warmup matmuls
N_WARM0 = int(os.environ.get("KERNEL_WARM0", "18"))
N_WARM1 = int(os.environ.get("KERNEL_WARM1", "18"))
N_WARMB = int(os.environ.get("KERNEL_WARMB", "4"))
N_WARMT = int(os.environ.get("KERNEL_WARMT", "14"))

# 1/16 scale on the AV ones column: r psum row = r/16, recip -> 16/r, hts
# = 16*h stays in e4m3 normal range, proj result scaled back by 1/16.
ONES_SC = 1.0 / 16.0
PROJ_SC = 1.0 / 16.0

# Schraudolph exp in e5m2-bit space: bits = int8(A*x + B); computes
# exp(x/8) for raw logits x.  A = 4/(8 ln2), B tuned for RNE convert.
SCHR8_A = 4.0 / (8.0 * np.log(2.0))
SCHR8_B = 15.0 * 4.0 - 0.2
# which (j, st) exp tiles run on ScalarE (table exp) vs DVE (Schraudolph).
# Both engines must stay busy within EACH j-phase; ~10/6 ACT/DVE balances
# the engines' fixed work.  Tunable: KERNEL_ACT_TILES="j:st,j:st,..."
_env = os.environ.get("KERNEL_ACT_TILES")
if _env:
    ACT_TILES = {0: set(), 1: set()}
    for tok in _env.split(","):
        j, st = tok.split(":")
        ACT_TILES[int(j)].add(int(st))
else:
    ACT_TILES = {0: {0, 2, 4, 6, 7}, 1: {1, 3, 5, 7}}

# packed f32 const layout (small: biases + gn affine)
CP_QKVB = 0          # qkvb: [128, 8]    (bias col per qk out tile)
CP_PROJB = 8         # projb: [128, 4]
CP_GNG = 12          # gng: [128, 4]
CP_GNB = 16          # gnb: [128, 4]
CP_VB = 20           # vbias broadcast: [128, 8*64] (only packed if v_bias)
CPF_COLS_NOV = 20
CPF_COLS_V = 20 + C
# packed bf16 const layout (groupnorm one-hot matmul operands)
CB_GM = 0            # gm: [128, 4*32]   (group one-hot, per ct)
CB_EM = 128          # em: [32, 4*128]   (rows 0:32; transpose of gm)
CB_COLS = 128 + 4 * 128

WP_OFF = 3 * C       # proj cols start
WPACK_COLS = 3 * C + C  # 2048


def build(num_devices=NCORES, q_bias=False, v_bias=False, p_bias=False):
    nc = bacc.Bacc("TRN2", target_bir_lowering=False, debug=False,
                   num_devices=num_devices)

    x_d = nc.dram_tensor("x", [BPC, C, HW], BF16, kind="ExternalInput").ap()
    # [128, pair, dual(kt), col] e4m3: contraction row = (2p+k)*128 + i
    wpack_d = nc.dram_tensor("wpack", [128, NP, 2, WPACK_COLS], F8E4,
                             kind="ExternalInput").ap()
    cpf_cols = CPF_COLS_V if v_bias else CPF_COLS_NOV
    cpack_d = nc.dram_tensor("cpack", [128, cpf_cols], F32,
                             kind="ExternalInput").ap()
    cpb_d = nc.dram_tensor("cpb", [128, CB_COLS], BF16,
                           kind="ExternalInput").ap()
    out_d = nc.dram_tensor("out", [BPC, C, HW], BF16, kind="ExternalOutput").ap()

    with tile.TileContext(nc) as tc:
        _body(tc, nc, x_d, wpack_d, cpack_d, cpb_d, out_d, q_bias, v_bias, p_bias)
    nc.compile()
    return nc


def _body(tc, nc, x_d, wpack_d, cpack_d, cpb_d, out_d, q_bias, v_bias, p_bias):
    from contextlib import ExitStack
    ctx = ExitStack()
    with ctx:
        const = ctx.enter_context(tc.tile_pool(name="const", bufs=1))
        xpool = ctx.enter_context(tc.tile_pool(name="xpool", bufs=2))
        xnpool = ctx.enter_context(tc.tile_pool(name="xnpool", bufs=2 * NP))
        qkvpool = ctx.enter_context(tc.tile_pool(name="qkvpool", bufs=CT))
        vtapool = ctx.enter_context(tc.tile_pool(name="vtapool", bufs=ST))
        exppool = ctx.enter_context(tc.tile_pool(name="exppool", bufs=6))
        hpool = ctx.enter_context(tc.tile_pool(name="hpool", bufs=2 * NP))
        hupool = ctx.enter_context(tc.tile_pool(name="hupool", bufs=4))
        rbpool = ctx.enter_context(tc.tile_pool(name="rbpool", bufs=2))
        ypool = ctx.enter_context(tc.tile_pool(name="ypool", bufs=2))
        smalls = ctx.enter_context(tc.tile_pool(name="smalls", bufs=6))
        rsm = ctx.enter_context(tc.tile_pool(name="rsm", bufs=2))
        kppool = ctx.enter_context(tc.tile_pool(name="kppool", bufs=1))
        drams = ctx.enter_context(tc.tile_pool(name="drams", bufs=4, space="DRAM"))
        # 3 two-bank S/filler slots + 1 two-bank AV accumulator = 8 banks.
        # One AV accumulator suffices because each pair runs its two heads
        # as sequential j-phases; 3 full-width S slots let the exps run as
        # single [128,1024] ops (half the instruction+semaphore overhead of
        # [128,512] chunks).
        ps_s = ctx.enter_context(tc.tile_pool(name="ps_s", bufs=3, space="PSUM"))
        ps_av = ctx.enter_context(tc.tile_pool(name="ps_av", bufs=1, space="PSUM"))

        def pslot(name):
            return ps_s.tile([128, HW], F32, tag="pss", name=name)

        # ---- batched input DMAs ----
        x_sb = [xpool.tile([128, CT, HW], BF16, tag="x", name=f"x_{b}")
                for b in range(BPC)]
        cp = const.tile([128, CPF_COLS_V if v_bias else CPF_COLS_NOV], F32)
        cb = const.tile([128, CB_COLS], BF16)
        wp = const.tile([128, NP, 2, WPACK_COLS], F8E4)

        def dma_x2(b, half):
            # two cts per dma_start (fewer ~0.6us Sync issue slots)
            xv = x_d[b].rearrange("(t p) w -> p t w", p=128)
            nc.sync.dma_start(out=x_sb[b][:, 2 * half:2 * half + 2, :],
                              in_=xv[:, 2 * half:2 * half + 2, :])

        # ALL of x0 ahead of everything else: the gn chain is gated by the
        # LAST ct's arrival (cpf/cpb in the middle of the x0 stream pushed
        # ct3 to ~20us)
        xv0 = x_d[0].rearrange("(t p) w -> p t w", p=128)
        for ct in range(CT):
            nc.sync.dma_start(out=x_sb[0][:, ct, :], in_=xv0[:, ct, :])
        nc.sync.dma_start(out=cp, in_=cpack_d)
        nc.sync.dma_start(out=cb, in_=cpb_d)
        # v-cols first (vt runs first), then the two qk column blocks the
        # first pair needs (ot0/ot4: waiting on the FULL wqk slice left the
        # PE sparse right after vt and re-engaged the HAM throttle), then
        # the rest, interleaved with x1
        nc.sync.dma_start(out=wp[:, :, :, 2 * C:3 * C],
                          in_=wpack_d[:, :, :, 2 * C:3 * C])
        nc.sync.dma_start(out=wp[:, :, :, 0:128], in_=wpack_d[:, :, :, 0:128])
        nc.sync.dma_start(out=wp[:, :, :, 512:640], in_=wpack_d[:, :, :, 512:640])
        dma_x2(1, 0)
        nc.sync.dma_start(out=wp[:, :, :, 128:512], in_=wpack_d[:, :, :, 128:512])
        nc.sync.dma_start(out=wp[:, :, :, 640:2 * C], in_=wpack_d[:, :, :, 640:2 * C])
        dma_x2(1, 1)
        nc.sync.dma_start(out=wp[:, :, :, WP_OFF:], in_=wpack_d[:, :, :, WP_OFF:])

        magic = const.tile([NUM_GROUPS, 1], U32)
        nc.gpsimd.memset(magic, 0x5F3759DF)
        ones33 = const.tile([33, HD], BF16)
        nc.gpsimd.memset(ones33, 1.0)

        # ---- HAM warmup: keep the PE active (and the clock un-gated)
        # through the DMA/groupnorm head so vt/qk start at 2.4GHz ----
        wu = None
        if USE_WARM:
            wu = const.tile([128, 512], BF16)
            nc.gpsimd.memset(wu, 0.0)

        def warmup(n):
            if wu is None:
                return
            for i in range(n):
                t = pslot("wup")
                nc.tensor.matmul(t[:, 0:512], lhsT=wu[:, 0:128],
                                 rhs=wu[:, 0:512], start=True, stop=True)

        warmup(N_WARM0 if USE_WARM else 0)

        def w8(p, c0, c1):
            return wp[:, p, :, c0:c1]

        state = [dict() for _ in range(BPC)]

        def emit_gn(b, split=False, warm_mid=0):
            emit_gn_stats(b)
            # the stats psum matmul FIFO-blocks the PE while waiting on the
            # DVE moment chain; warmups emitted here fill the window between
            # it and the (also-blocking) apply matmuls so HAM stays warm
            warmup(warm_mid)
            emit_gn_apply(b, split=split)

        def emit_gn_stats_a(b, cts):
            """bn stats + per-ct channel moments (DVE)."""
            s = state[b]
            cm2s = s.setdefault("cm2s", [None] * CT)
            for ct in cts:
                stats = smalls.tile([128, 2, 6], F32, tag="bnst", name=f"bnst_{b}_{ct}")
                for sg in range(2):
                    nc.vector.bn_stats(out=stats[:, sg, :],
                                       in_=x_sb[b][:, ct, sg * 512:(sg + 1) * 512])
                cmv = smalls.tile([128, 2], F32, tag="cmv", name=f"cmv_{b}_{ct}")
                nc.vector.bn_aggr(out=cmv, in_=stats)
                cm2 = smalls.tile([128, 2], BF16, tag="cm2", name=f"cm2_{b}_{ct}")
                nc.vector.tensor_copy(out=cm2[:, 0:1], in_=cmv[:, 0:1])
                nc.vector.tensor_tensor(out=cm2[:, 1:2], in0=cmv[:, 0:1], in1=cmv[:, 0:1], op=ALU.mult)
                nc.vector.tensor_tensor(out=cm2[:, 1:2], in0=cm2[:, 1:2], in1=cmv[:, 1:2], op=ALU.add)
                cm2s[ct] = cm2

        def emit_gn_stats(b):
            """group-norm stats of x_sb[b]: bn stats + group reduce + quake
            rstd."""
            s = state[b]
            todo = [ct for ct in range(CT)
                    if s.get("cm2s") is None or s["cm2s"][ct] is None]
            if todo:
                emit_gn_stats_a(b, todo)
            cm2s = s["cm2s"]
            ps_g = pslot(f"psg_{b}")
            for ct in range(CT):
                nc.tensor.matmul(ps_g[0:NUM_GROUPS, 0:2],
                                 lhsT=cb[:, CB_GM + ct * 32:CB_GM + (ct + 1) * 32],
                                 rhs=cm2s[ct], start=(ct == 0), stop=(ct == CT - 1))
            gstat = smalls.tile([NUM_GROUPS, 2], BF16, tag="gstat", name=f"gstat_{b}")
            nc.vector.tensor_scalar_mul(out=gstat, in0=ps_g[0:NUM_GROUPS, 0:2], scalar1=1.0 / GS)
            var_g = smalls.tile([NUM_GROUPS, 1], F32, tag="varg", name=f"varg_{b}")
            nc.vector.tensor_tensor(out=var_g, in0=gstat[:, 0:1], in1=gstat[:, 0:1], op=ALU.mult)
            nc.vector.tensor_tensor(out=var_g, in0=gstat[:, 1:2], in1=var_g, op=ALU.subtract)
            nc.vector.tensor_scalar_add(out=var_g, in0=var_g, scalar1=EPS)
            y_n = smalls.tile([NUM_GROUPS, 1], F32, tag="yn", name=f"yn_{b}")
            t_n = smalls.tile([NUM_GROUPS, 1], F32, tag="tn", name=f"tn_{b}")
            nc.vector.tensor_scalar(out=y_n.bitcast(U32), in0=var_g.bitcast(U32),
                                    scalar1=1, scalar2=None, op0=ALU.logical_shift_right)
            nc.vector.tensor_tensor(out=y_n.bitcast(U32), in0=magic,
                                    in1=y_n.bitcast(U32), op=ALU.subtract)
            for _ in range(2):
                nc.vector.tensor_tensor(out=t_n, in0=y_n, in1=y_n, op=ALU.mult)
                nc.vector.scalar_tensor_tensor(out=t_n, in0=t_n, scalar=-0.5,
                                               in1=var_g, op0=ALU.mult, op1=ALU.mult)
                nc.vector.scalar_tensor_tensor(out=y_n, in0=t_n, scalar=1.5,
                                               in1=y_n, op0=ALU.add, op1=ALU.mult)
            nc.vector.tensor_copy(out=gstat[:, 1:2], in_=y_n)
            s["gstat"] = gstat

        def emit_gn_apply(b, split=False):
            s = state[b]
            gstat = s["gstat"]
            xns = [xnpool.tile([128, 2, HW], F8E4, tag="xn", name=f"xn_{b}_{p}")
                   for p in range(NP)]
            for ct in range(CT):
                ps_e = pslot(f"pse_{b}_{ct}")
                nc.tensor.matmul(ps_e[:, 0:2],
                                 lhsT=cb[0:32, CB_EM + ct * 128:CB_EM + (ct + 1) * 128],
                                 rhs=gstat, start=True, stop=True)
                sc = smalls.tile([128, 1], F32, tag="sc", name=f"sc_{b}_{ct}")
                bi = smalls.tile([128, 1], F32, tag="bi", name=f"bi_{b}_{ct}")
                nc.vector.tensor_tensor(out=sc, in0=cp[:, CP_GNG + ct:CP_GNG + ct + 1],
                                        in1=ps_e[:, 1:2], op=ALU.mult)
                nc.vector.tensor_tensor(out=bi, in0=ps_e[:, 0:1], in1=sc, op=ALU.mult)
                nc.vector.tensor_tensor(out=bi, in0=cp[:, CP_GNB + ct:CP_GNB + ct + 1],
                                        in1=bi, op=ALU.subtract)
                # xn is on the critical path for vt/qk: split DVE/GpSimd
                if split:
                    xeng = nc.gpsimd if (ct == 3 and USE_GP) else nc.vector
                else:
                    xeng = nc.gpsimd if (ct >= 2 and USE_GP) else nc.vector
                xeng.tensor_scalar(out=xns[ct // 2][:, ct % 2, :],
                                   in0=x_sb[b][:, ct, :],
                                   scalar1=sc, scalar2=bi, op0=ALU.mult, op1=ALU.add)
            s["xns"] = xns

        def emit_vt(b, sts):
            """vT[s, o] = sum_c xn[c, s] * WvT[c, o] (fp8 DR over kt pairs);
            + 1/16 col at 64 for the scaled rowsum."""
            s = state[b]
            for st in sts:
                ps_v = pslot(f"psv_{b}_{st}")
                for p in range(NP):
                    nc.tensor.matmul(ps_v[:, 0:C],
                                     lhsT=s["xns"][p][:, :, st * 128:(st + 1) * 128],
                                     rhs=w8(p, 2 * C, 3 * C),
                                     start=(p == 0), stop=(p == NP - 1),
                                     perf_mode=DR)
                pv = ps_v[:, 0:C].rearrange("p (h d) -> p h d", h=NUM_HEADS)
                vtaps = s.setdefault("vtaps", [None] * (ST // 2))
                sp, k = st // 2, st % 2
                if vtaps[sp] is None:
                    # head stride 66 (528B) keeps the DoubleRow dual-
                    # subtile step 16B-aligned
                    vtaps[sp] = vtapool.tile([128, 2, NUM_HEADS, HD + 2], F8E4,
                                             tag="vta", name=f"vta_{b}_{sp}")
                    nc.gpsimd.memset(vtaps[sp][:, :, :, HD:HD + 1], ONES_SC)
                dst = vtaps[sp][:, k, :, 0:HD]
                if v_bias:
                    nc.vector.tensor_tensor(
                        out=dst, in0=pv,
                        in1=cp[:, CP_VB:CP_VB + C].rearrange("p (h d) -> p h d", h=NUM_HEADS),
                        op=ALU.add)
                else:
                    nc.vector.tensor_copy(out=dst, in_=pv)

        def emit_qk(b, ots):
            """q/k channel-major (fp8 DR); q drains on ScalarE, padded k
            halves on DVE."""
            s = state[b]
            qks = s.setdefault("qks", [None] * QKT)
            for ot in ots:
                ps_q = pslot(f"psq_{b}_{ot}")
                for p in range(NP):
                    for nh in range(NH):
                        nc.tensor.matmul(ps_q[:, nh * 512:(nh + 1) * 512],
                                         lhsT=w8(p, ot * 128, (ot + 1) * 128),
                                         rhs=s["xns"][p][:, :, nh * 512:(nh + 1) * 512],
                                         start=(p == 0), stop=(p == NP - 1),
                                         perf_mode=DR)
                if ot >= CT:
                    kps = s.setdefault("kps", {})
                    pads = []
                    for j in range(2):
                        # head j's k rows stay at partitions 64j:64j+64 (same
                        # rows its q occupies in the rhs); other half is zero.
                        # tag-stable slot: the zero half written for b=0
                        # persists physically for b=1's reuse.
                        kp = kppool.tile([128, HW], BF16, tag=f"kp{(ot - CT) * 2 + j}",
                                         name=f"kp_{b}_{ot}_{j}")
                        lo, hi = 64 * j, 64 * j + 64
                        if b == 0:
                            nc.gpsimd.memset(kp[0:64, :] if j else kp[64:128, :], 0.0)
                        dst = kp[lo:hi, :]
                        if q_bias:
                            nc.vector.tensor_scalar_add(
                                out=dst, in0=ps_q[lo:hi, :],
                                scalar1=cp[:, CP_QKVB + ot:CP_QKVB + ot + 1][lo:hi])
                        elif b == 0:
                            nc.scalar.activation(out=dst, in_=ps_q[lo:hi, :],
                                                 func=ACTF.Copy)
                        else:
                            nc.vector.tensor_copy(out=dst, in_=ps_q[lo:hi, :])
                        pads.append(kp)
                    kps[ot - CT] = pads
                    continue
                qt = qkvpool.tile([128, HW], BF16, tag="qkv", name=f"qk_{b}_{ot}")
                if q_bias:
                    nc.vector.tensor_scalar_add(out=qt, in0=ps_q,
                                                scalar1=cp[:, CP_QKVB + ot:CP_QKVB + ot + 1])
                else:
                    nc.scalar.activation(out=qt, in_=ps_q, func=ACTF.Copy)
                qks[ot] = qt

        def emit_pair(b, hp, last=False):
            """Head pair (2hp, 2hp+1) as two sequential j-phases sharing one
            AV accumulator.  Full-width S slots (3-deep rotation) -> exps are
            single [128,1024] ops split ACT (table exp) / DVE (Schraudolph)."""
            s = state[b]
            if "hts" not in s:
                s["hts"] = [hpool.tile([128, 2, HW], F8E4, tag="hm",
                                       name=f"hm_{b}_{p}") for p in range(NP)]
            qt2 = s["qks"][hp]
            kpads = s["kps"][hp]
            vtaps = s["vtaps"]
            p, kk = hp // 2, hp % 2
            hus = []
            r2 = rsm.tile([33, HW], F32, tag="r2", name=f"r2_{b}_{hp}")
            ri2 = rsm.tile([33, HW], F32, tag="ri2", name=f"ri2_{b}_{hp}")
            ri2b = None
            if last and not SAFE_LAST:
                ri2b = rsm.tile([33, HW], BF16, tag="ri2b", name=f"ri2b_{b}_{hp}")

            all_slots = [{}, {}]
            all_pexps = [{}, {}]
            ps_os = [None, None]

            def s_mm(j, st):
                t = pslot(f"pss_{b}_{hp}_{j}_{st}")
                # K=128 with zero rows in the k operand: rows of q
                # belonging to the other head hit zero weights
                for nh in range(NH):
                    nc.tensor.matmul(t[:, nh * 512:(nh + 1) * 512],
                                     lhsT=kpads[j][:, st * 128:(st + 1) * 128],
                                     rhs=qt2[:, nh * 512:(nh + 1) * 512],
                                     start=True, stop=True)
                all_slots[j][st] = t

            def emit_exp(j, st):
                sp, k = st // 2, st % 2
                pexps = all_pexps[j]
                if k == 0:
                    pexps[sp] = exppool.tile([128, 2, HW], F8E5, tag="pexp",
                                             name=f"pexp_{b}_{hp}_{j}_{sp}")
                dst = pexps[sp][:, k, :]
                # very first phase: DVE still has the b1-stats/cast backlog
                # queued ahead of its exps, which would stall the S-slot
                # rotation -> run every exp of this one phase on ACT
                first_phase = (b == 0 and hp == 0 and j == 0)
                if st not in ACT_TILES[j] and not first_phase:
                    nc.vector.tensor_scalar(out=dst.bitcast(I8),
                                            in0=all_slots[j][st],
                                            scalar1=SCHR8_A, scalar2=SCHR8_B,
                                            op0=ALU.mult, op1=ALU.add)
                else:
                    nc.scalar.activation(out=dst, in_=all_slots[j][st],
                                         func=ACTF.Exp, scale=1.0 / np.sqrt(HD))

            def av_block(j):
                # ALL AV matmuls after the S stream: an AV matmul waiting on
                # an exp would head-of-line block the S matmuls queued
                # behind it (PE FIFO), serializing the phase to the exp chain
                h = 2 * hp + j
                for sp in range(ST // 2):
                    for nh in range(NH):
                        nc.tensor.matmul(
                            ps_os[j][0:HD + 1, nh * 512:(nh + 1) * 512],
                            lhsT=vtaps[sp][:, :, h, 0:HD + 1],
                            rhs=all_pexps[j][sp][:, :, nh * 512:(nh + 1) * 512],
                            start=(sp == 0), stop=(sp == ST // 2 - 1),
                            perf_mode=DR)

            def drain(j):
                # frees the single AV accumulator for the next phase; r rows
                # bounce via SBUF (reciprocal_approx_fast cannot read PSUM)
                ps_o = ps_os[j]
                hu = hupool.tile([HD, HW], BF16, tag="hu", name=f"hu_{b}_{hp}_{j}")
                nc.scalar.activation(out=hu, in_=ps_o[0:HD, :], func=ACTF.Copy)
                hus.append(hu)
                nc.scalar.activation(out=r2[32 * j:32 * j + 1, :],
                                     in_=ps_o[HD:HD + 1, :], func=ACTF.Copy)
                if not (last and not SAFE_LAST):
                    return
                nc.vector.reciprocal_approx_fast(
                    out=ri2[32 * j:32 * j + 1, :],
                    in_=r2[32 * j:32 * j + 1, :])
                if j == 0:
                    # j0 normalizes via the DMA roundtrip (all SBUF-side)
                    # so the single AV accumulator frees for j1's phase
                    # right after the hu/r copies
                    rs = drams.tile([2, HW], F32, tag="rs", name=f"rs_{b}_{hp}")
                    nc.sync.dma_start(out=rs[0:1, :], in_=ri2[0:1, :])
                    rb = rbpool.tile([HD, 2, HW], F32, tag="rb", name=f"rb_{b}_{hp}")
                    rs_bc = bass.AP(tensor=rs.tensor, offset=rs.offset,
                                    ap=[[0, HD]] + [list(rs.ap)[-1]])
                    nc.sync.dma_start(out=rb[:, 0, :], in_=rs_bc)
                    eng0 = nc.gpsimd if USE_GP else nc.vector
                    eng0.tensor_tensor(out=s["hts"][p][0:64, kk, :],
                                       in0=hu, in1=rb[:, 0, :], op=ALU.mult)
                else:
                    # j1 (the true tail): K=1 bf16 broadcast matmul into
                    # rows 64:128 of the drained AV psum + DVE multiply
                    nc.vector.tensor_copy(out=ri2b[32:33, :], in_=ri2[32:33, :])
                    for nh in range(NH):
                        nc.tensor.matmul(ps_os[1][HD:128, nh * 512:(nh + 1) * 512],
                                         lhsT=ones33[32:33, :],
                                         rhs=ri2b[32:33, nh * 512:(nh + 1) * 512],
                                         start=True, stop=True)
                    nc.vector.tensor_tensor(out=s["hts"][p][64:128, kk, :],
                                            in0=hu, in1=ps_os[1][HD:128, :],
                                            op=ALU.mult)

            # phase j0
            ps_os[0] = ps_av.tile([128, HW], F32, tag="psav", name=f"pso_{b}_{hp}_0")
            s_mm(0, 0)
            for st in range(ST):
                emit_exp(0, st)
                if st + 1 < ST:
                    s_mm(0, st + 1)
            # prefill j1's S pipeline before the AV-j0 block so the PE keeps
            # streaming while AV waits on j0's exp tail; j0's drain copies go
            # after j1's first exps so they don't head-of-line block the
            # ACT/DVE queues
            ps_os[1] = ps_av.tile([128, HW], F32, tag="psav", name=f"pso_{b}_{hp}_1")
            s_mm(1, 0)
            s_mm(1, 1)
            av_block(0)
            emit_exp(1, 0)
            s_mm(1, 2)
            emit_exp(1, 1)
            s_mm(1, 3)
            drain(0)
            for st in range(2, ST):
                emit_exp(1, st)
                if st + 2 < ST:
                    s_mm(1, st + 2)
            av_block(1)
            drain(1)

            if last and not SAFE_LAST:
                return None
            nc.vector.reciprocal_approx_fast(out=ri2, in_=r2)
            rs = drams.tile([2, HW], F32, tag="rs", name=f"rs_{b}_{hp}")
            for j in range(2):
                nc.sync.dma_start(out=rs[j:j + 1, :], in_=ri2[32 * j:32 * j + 1, :])
            rb = rbpool.tile([HD, 2, HW], F32, tag="rb", name=f"rb_{b}_{hp}")
            rs_bc = bass.AP(tensor=rs.tensor, offset=rs.offset,
                            ap=[[0, HD]] + list(rs.ap))
            nc.sync.dma_start(out=rb, in_=rs_bc)

            eng = nc.gpsimd if USE_GP else nc.vector

            def finish():
                for j in range(2):
                    eng.tensor_tensor(out=s["hts"][p][j * 64:j * 64 + 64, kk, :],
                                      in0=hus[j], in1=rb[:, j, :], op=ALU.mult)
            return finish

        def emit_proj(b, ots):
            s = state[b]
            for ot in ots:
                yt = ypool.tile([128, HW], BF16, tag="yt", name=f"yt_{b}_{ot}")
                ps_p = pslot(f"psp_{b}_{ot}")
                for p in range(NP):
                    for nh in range(NH):
                        nc.tensor.matmul(ps_p[:, nh * 512:(nh + 1) * 512],
                                         lhsT=w8(p, WP_OFF + ot * 128, WP_OFF + (ot + 1) * 128),
                                         rhs=s["hts"][p][:, :, nh * 512:(nh + 1) * 512],
                                         start=(p == 0), stop=(p == NP - 1),
                                         perf_mode=DR)
                xin = x_sb[b][:, ot, :]
                if p_bias:
                    nc.vector.tensor_scalar(out=yt, in0=ps_p, scalar1=PROJ_SC,
                                            scalar2=cp[:, CP_PROJB + ot:CP_PROJB + ot + 1],
                                            op0=ALU.mult, op1=ALU.add)
                    nc.vector.tensor_tensor(out=yt, in0=yt, in1=xin, op=ALU.add)
                elif b == 1 and ot % 2 == 1:
                    # tail drains: ACT scaled-copy + GpSimd residual add
                    # (ACT/GpSimd are idle at the tail; the 4 serial DVE
                    # drains were the last ~5us of the kernel)
                    yh = ypool.tile([128, HW], BF16, tag="yh", name=f"yh_{b}_{ot}")
                    nc.scalar.activation(out=yh, in_=ps_p, func=ACTF.Copy,
                                         scale=PROJ_SC)
                    nc.gpsimd.tensor_tensor(out=yt, in0=yh, in1=xin, op=ALU.add)
                else:
                    nc.vector.scalar_tensor_tensor(out=yt, in0=ps_p,
                                                   scalar=PROJ_SC, in1=xin,
                                                   op0=ALU.mult, op1=ALU.add)
                nc.sync.dma_start(out=out_d[b, ot * 128:(ot + 1) * 128, :], in_=yt)

        # ---- schedule ----
        emit_gn(0, split=True, warm_mid=8)
        warmup(N_WARM1)
        emit_vt(0, range(ST))
        emit_qk(0, [0, 4])
        f00 = emit_pair(0, 0)
        emit_qk(0, [1, 5])
        emit_gn_stats_a(1, [0, 1])
        warmup(N_WARMB)
        f01 = emit_pair(0, 1)
        emit_qk(0, [2, 6])
        emit_gn_stats(1)
        emit_gn_apply(1)
        f00()
        warmup(N_WARMB)
        f02 = emit_pair(0, 2)
        emit_qk(0, [3, 7])
        emit_vt(1, range(0, 4))
        f01()
        warmup(N_WARMB)
        f03 = emit_pair(0, 3)
        emit_vt(1, range(4, ST))
        emit_qk(1, [0, 4])
        f02()
        warmup(N_WARMB)
        f10 = emit_pair(1, 0)
        emit_qk(1, [1, 5, 2, 6])
        f03()
        emit_proj(0, [0, 1])
        warmup(N_WARMB)
        f11 = emit_pair(1, 1)
        f10()
        emit_qk(1, [3, 7])
        warmup(N_WARMB)
        f12 = emit_pair(1, 2)
        f11()
        emit_proj(0, [2, 3])
        f12()
        warmup(N_WARMB)
        f13 = emit_pair(1, 3, last=True)
        if f13 is not None:
            f13()
        # keep the PE active through the last pair's serial normalize chain:
        # a rethrottle here made ALL 16 proj matmuls run at 1.2GHz
        warmup(N_WARMT)
        emit_proj(1, range(CT))


def make_host_inputs(x, gn_gamma, gn_beta, qkv_w, qkv_b, proj_w, proj_b):
    """Full inputs -> list of per-core in_maps (packed weight/const tensors)."""
    x = np.asarray(x, dtype=np.float32).reshape(B, C, HW).astype(ml_dtypes.bfloat16)
    wqkvT = np.asarray(qkv_w, dtype=np.float32).T          # [C, 3C]
    wprojT = np.asarray(proj_w, dtype=np.float32).T        # [C, C]
    wall = np.concatenate([wqkvT, wprojT], axis=1)         # [C, 4C]
    # TRN e4m3 tops out at 240 (not OCP's 448)
    w8 = np.clip(wall, -240.0, 240.0).astype(ml_dtypes.float8_e4m3fn)
    wpack = np.zeros((128, NP, 2, WPACK_COLS), dtype=ml_dtypes.float8_e4m3fn)
    for p in range(NP):
        for k in range(2):
            kt = 2 * p + k
            wpack[:, p, k, :] = w8[kt * 128:(kt + 1) * 128, :]

    qkv_b = np.asarray(qkv_b, dtype=np.float32)
    v_bias = bool(np.any(qkv_b[2 * C:]))
    cpack = np.zeros((128, CPF_COLS_V if v_bias else CPF_COLS_NOV),
                     dtype=np.float32)
    for ot in range(QKT):
        cpack[:, CP_QKVB + ot] = qkv_b[ot * 128:(ot + 1) * 128]
    for t in range(CT):
        cpack[:, CP_PROJB + t] = np.asarray(proj_b, dtype=np.float32)[t * 128:(t + 1) * 128]
        cpack[:, CP_GNG + t] = np.asarray(gn_gamma, dtype=np.float32)[t * 128:(t + 1) * 128]
        cpack[:, CP_GNB + t] = np.asarray(gn_beta, dtype=np.float32)[t * 128:(t + 1) * 128]
    if v_bias:
        cpack[:, CP_VB:CP_VB + C] = qkv_b[2 * C:3 * C][None, :]

    cpb = np.zeros((128, CB_COLS), dtype=ml_dtypes.bfloat16)
    for t in range(CT):
        for k in range(128):
            cpb[k, CB_GM + t * 32 + (t * 128 + k) // GS] = 1.0
            cpb[(t * 128 + k) // GS, CB_EM + t * 128 + k] = 1.0

    shared = {"wpack": wpack, "cpack": cpack, "cpb": cpb}
    return [dict(shared, x=np.ascontiguousarray(x[i * BPC:(i + 1) * BPC]))
            for i in range(NCORES)]


_NC_CACHE = {}


def _get_nc(q_bias=False, v_bias=False, p_bias=False):
    key = (q_bias, v_bias, p_bias)
    if key not in _NC_CACHE:
        _NC_CACHE[key] = build(q_bias=q_bias, v_bias=v_bias, p_bias=p_bias)
    return _NC_CACHE[key]


def kernel(x, gn_gamma, gn_beta, qkv_w, qkv_b, proj_w, proj_b):
    from concourse.bass_utils import run_bass_kernel_spmd
    qkv_b = np.asarray(qkv_b)
    nc = _get_nc(q_bias=bool(np.any(qkv_b[:2 * C])),
                 v_bias=bool(np.any(qkv_b[2 * C:])),
                 p_bias=bool(np.any(np.asarray(proj_b))))
    in_maps = make_host_inputs(x, gn_gamma, gn_beta, qkv_w, qkv_b, proj_w, proj_b)
    res = run_bass_kernel_spmd(nc, in_maps, list(range(NCORES)))
    out = np.concatenate([res.results[i]["out"] for i in range(NCORES)], axis=0)
    return out.reshape(B, C, H, W).astype(np.float32)


# revision 73
# speedup vs baseline: 1.0213x; 1.0213x over previous
"""AttentionBlock (GroupNorm -> qkv -> MHA -> proj -> residual) on 8 trn2 cores.

Data-parallel over batch: 16 batches -> 2 per core. No collectives.

Per-core math (per batch item, c=512 channels, hw=1024 spatial, 8 heads x 64):
  xn = groupnorm(x)                     [c, hw] layout, stored e4m3 DR pairs
  q,k = Wqk^T.T @ xn (fp8 DoubleRow)    [2c, hw] -> bf16 SBUF (S path)
  vT  = xn.T @ WvT (fp8 DR)             [hw, c] e4m3 (+ 1/16 ones col)
  per head: S^T = kpad^T q (bf16 K=128) [s, t]
            P = exp(S^T / 8)            e5m2 (ACT table exp / DVE Schraudolph)
            AV fp8 DR K=256             rows 0..64 unnorm out, row 64 = r/16
            h*16 = AV[0:64] * (16/r)    e4m3 hts
  y = x + (WprojT.T @ hts)(fp8 DR)/16

vs the 257us bf16 version (this file's ancestor), trace-driven changes
(257 -> ~204us measured; engine totals ACT ~123 / DVE ~135 / PE ~148us):
  - fp8 DoubleRow for ALL K>=256 matmuls (qkv, vt, proj): xn/weights/hts in
    e4m3, contraction pairs over kt tiles (dual-subtile middle dim on a
    [128, pair, 2, col] wpack).  Halves those phases' PE instructions.
    Errors are diluted ~16x because ||h_attn|| ~ 0.07*||x|| (softmax
    averaging): measured rel err 7.5e-3 vs 3.6e-3 for the bf16 ancestor.
  - pairs run as two sequential j-phases sharing ONE [128,1024] AV psum
    accumulator (2 banks), freeing 6 banks for THREE full-width S slots ->
    exps are single [128,1024] ops (vs [128,512] chunks: half the
    instruction + semaphore overhead).  Phase st-cadence is ~1.0-1.3us.
  - ALL AV matmuls emitted after the phase's S stream (an AV waiting on an
    exp head-of-line blocks the S matmuls behind it in the PE FIFO); j1's
    first two S matmuls emitted before the AV-j0 block; j0's drain copies
    emitted after j1's first exps (ACT FIFO would block them).
  - exp engine split per (j, st): ACT_TILES on table-exp ScalarE, rest on
    DVE via Schraudolph e5m2 bit-affine (9/7 split balances ACT vs DVE
    incl. their drain work).
  - x shipped bf16 (DMA fabric here moves ~50-80GB/s effective; f32 x was
    gating groupnorm stats at ~28us), two cts per dma_start (each start
    costs ~0.6us of Sync issue time); gm/em one-hots shipped bf16 in a
    separate cpb (f32 one-hots made the gn matmuls run 2-pass fp32 mode).
  - HAM warmup matmuls bridge t=0 -> xn-ready (~18us): without them the
    whole vt/qk phase of b=0 ran at 1.2GHz.  WARM0 before gn(0), WARM1
    after (tuned empirically; more is NOT better - they delay real work).
  - softmax denominators: r rows ACT-copied to SBUF (reciprocal_approx_fast
    returns garbage on PSUM inputs - hardware-verified), one batched recip,
    f32 DMA broadcast roundtrip, deferred GpSimd hts multiplies.  Last
    pair: j0 via the roundtrip (frees the single AV accumulator fast), j1
    via a K=1 bf16 ones-matmul broadcast + DVE multiply (fp32 matmuls here
    run 2-pass LOW/HIGH at ~1.1us per MM - use bf16).
  - gn stats(1) split across two pair boundaries; b1 stats/apply emitted
    mid-pair-region where DVE has slack.
  - memsets on GpSimd; zero-padded k for S (HAM clock gate; note the pad
    costs NO PE cycles - matmul time is N columns regardless of K) and the
    e5m2 Schraudolph exp kept from the ancestor.
"""

import os

import numpy as np
import ml_dtypes

import concourse.bass as bass
import concourse.tile as tile
import concourse.mybir as mybir
from concourse import bacc

NUM_HEADS = 8
NUM_GROUPS = 32
EPS = 1e-5
B, C, H, W = 16, 512, 32, 32
HW = H * W                  # 1024
NCORES = 8
BPC = B // NCORES           # 2 batches per core
HD = C // NUM_HEADS         # 64
GS = C // NUM_GROUPS        # 16 channels per group
CT = C // 128               # 4 channel tiles
NP = CT // 2                # 2 channel-tile pairs (fp8 DR contraction)
QKT = 2 * C // 128          # 8 q+k output tiles
ST = HW // 128              # 8 sequence tiles
NH = HW // 512              # 2 moving-dim chunks of 512

F32 = mybir.dt.float32
BF16 = mybir.dt.bfloat16
F8E4 = mybir.dt.float8e4
F8E5 = mybir.dt.float8e5
I16 = mybir.dt.int16
I8 = mybir.dt.int8
U32 = mybir.dt.uint32
ALU = mybir.AluOpType
ACTF = mybir.ActivationFunctionType
DR = mybir.MatmulPerfMode.DoubleRow

USE_GP = os.environ.get("KERNEL_NO_GP") != "1"      # hts/xn work on gpsimd
# r rows must bounce through SBUF: reciprocal_approx_fast (custom DVE op)
# returns garbage on PSUM inputs (hardware-verified: direct-PSUM -> NaN out)
SAFE_R = os.environ.get("KERNEL_SAFE_R", "1") == "1"
SAFE_LAST = os.environ.get("KERNEL_SAFE_LAST") == "1"  # last pair uses DMA roundtrip
USE_WARM = os.environ.get("KERNEL_NO_WARM") != "1"  # HAM warmup matmuls
N_WARM0 = int(os.environ.get("KERNEL_WARM0", "18"))
N_WARM1 = int(os.environ.get("KERNEL_WARM1", "18"))
N_WARMB = int(os.environ.get("KERNEL_WARMB", "4"))
N_WARMT = int(os.environ.get("KERNEL_WARMT", "14"))

# 1/16 scale on the AV ones column: r psum row = r/16, recip -> 16/r, hts
# = 16*h stays in e4m3 normal range, proj result scaled back by 1/16.
ONES_SC = 1.0 / 16.0
PROJ_SC = 1.0 / 16.0

# Schraudolph exp in e5m2-bit space: bits = int8(A*x + B); computes
# exp(x/8) for raw logits x.  A = 4/(8 ln2), B tuned for RNE convert.
SCHR8_A = 4.0 / (8.0 * np.log(2.0))
SCHR8_B = 15.0 * 4.0 - 0.2
# which (j, st) exp tiles run on ScalarE (table exp) vs DVE (Schraudolph).
# Both engines must stay busy within EACH j-phase; ~10/6 ACT/DVE balances
# the engines' fixed work.  Tunable: KERNEL_ACT_TILES="j:st,j:st,..."
_env = os.environ.get("KERNEL_ACT_TILES")
if _env:
    ACT_TILES = {0: set(), 1: set()}
    for tok in _env.split(","):
        j, st = tok.split(":")
        ACT_TILES[int(j)].add(int(st))
else:
    ACT_TILES = {0: {0, 2, 4, 6, 7}, 1: {1, 3, 5, 7}}

# packed f32 const layout (small: biases + gn affine)
CP_QKVB = 0          # qkvb: [128, 8]    (bias col per qk out tile)
CP_PROJB = 8         # projb: [128, 4]
CP_GNG = 12          # gng: [128, 4]
CP_GNB = 16          # gnb: [128, 4]
CP_VB = 20           # vbias broadcast: [128, 8*64] (only packed if v_bias)
CPF_COLS_NOV = 20
CPF_COLS_V = 20 + C
# packed bf16 const layout (groupnorm one-hot matmul operands)
CB_GM = 0            # gm: [128, 4*32]   (group one-hot, per ct)
CB_EM = 128          # em: [32, 4*128]   (rows 0:32; transpose of gm)
CB_COLS = 128 + 4 * 128

WP_OFF = 3 * C       # proj cols start
WPACK_COLS = 3 * C + C  # 2048


def build(num_devices=NCORES, q_bias=False, v_bias=False, p_bias=False):
    nc = bacc.Bacc("TRN2", target_bir_lowering=False, debug=False,
                   num_devices=num_devices)

    x_d = nc.dram_tensor("x", [BPC, C, HW], BF16, kind="ExternalInput").ap()
    # [128, pair, dual(kt), col] e4m3: contraction row = (2p+k)*128 + i
    wpack_d = nc.dram_tensor("wpack", [128, NP, 2, WPACK_COLS], F8E4,
                             kind="ExternalInput").ap()
    cpf_cols = CPF_COLS_V if v_bias else CPF_COLS_NOV
    cpack_d = nc.dram_tensor("cpack", [128, cpf_cols], F32,
                             kind="ExternalInput").ap()
    cpb_d = nc.dram_tensor("cpb", [128, CB_COLS], BF16,
                           kind="ExternalInput").ap()
    out_d = nc.dram_tensor("out", [BPC, C, HW], BF16, kind="ExternalOutput").ap()

    with tile.TileContext(nc) as tc:
        _body(tc, nc, x_d, wpack_d, cpack_d, cpb_d, out_d, q_bias, v_bias, p_bias)
    nc.compile()
    return nc


def _body(tc, nc, x_d, wpack_d, cpack_d, cpb_d, out_d, q_bias, v_bias, p_bias):
    from contextlib import ExitStack
    ctx = ExitStack()
    with ctx:
        const = ctx.enter_context(tc.tile_pool(name="const", bufs=1))
        xpool = ctx.enter_context(tc.tile_pool(name="xpool", bufs=2))
        xnpool = ctx.enter_context(tc.tile_pool(name="xnpool", bufs=2 * NP))
        qkvpool = ctx.enter_context(tc.tile_pool(name="qkvpool", bufs=CT))
        vtapool = ctx.enter_context(tc.tile_pool(name="vtapool", bufs=ST))
        exppool = ctx.enter_context(tc.tile_pool(name="exppool", bufs=6))
        hpool = ctx.enter_context(tc.tile_pool(name="hpool", bufs=2 * NP))
        hupool = ctx.enter_context(tc.tile_pool(name="hupool", bufs=4))
        rbpool = ctx.enter_context(tc.tile_pool(name="rbpool", bufs=2))
        ypool = ctx.enter_context(tc.tile_pool(name="ypool", bufs=2))
        smalls = ctx.enter_context(tc.tile_pool(name="smalls", bufs=6))
        rsm = ctx.enter_context(tc.tile_pool(name="rsm", bufs=2))
        kppool = ctx.enter_context(tc.tile_pool(name="kppool", bufs=1))
        drams = ctx.enter_context(tc.tile_pool(name="drams", bufs=4, space="DRAM"))
        # 3 two-bank S/filler slots + 1 two-bank AV accumulator = 8 banks.
        # One AV accumulator suffices because each pair runs its two heads
        # as sequential j-phases; 3 full-width S slots let the exps run as
        # single [128,1024] ops (half the instruction+semaphore overhead of
        # [128,512] chunks).
        ps_s = ctx.enter_context(tc.tile_pool(name="ps_s", bufs=3, space="PSUM"))
        ps_av = ctx.enter_context(tc.tile_pool(name="ps_av", bufs=1, space="PSUM"))

        def pslot(name):
            return ps_s.tile([128, HW], F32, tag="pss", name=name)

        # ---- batched input DMAs ----
        x_sb = [xpool.tile([128, CT, HW], BF16, tag="x", name=f"x_{b}")
                for b in range(BPC)]
        cp = const.tile([128, CPF_COLS_V if v_bias else CPF_COLS_NOV], F32)
        cb = const.tile([128, CB_COLS], BF16)
        wp = const.tile([128, NP, 2, WPACK_COLS], F8E4)

        def dma_x2(b, half):
            # two cts per dma_start (fewer ~0.6us Sync issue slots)
            xv = x_d[b].rearrange("(t p) w -> p t w", p=128)
            nc.sync.dma_start(out=x_sb[b][:, 2 * half:2 * half + 2, :],
                              in_=xv[:, 2 * half:2 * half + 2, :])

        # ALL of x0 ahead of everything else: the gn chain is gated by the
        # LAST ct's arrival (cpf/cpb in the middle of the x0 stream pushed
        # ct3 to ~20us)
        xv0 = x_d[0].rearrange("(t p) w -> p t w", p=128)
        for ct in range(CT):
            nc.sync.dma_start(out=x_sb[0][:, ct, :], in_=xv0[:, ct, :])
        nc.sync.dma_start(out=cp, in_=cpack_d)
        nc.sync.dma_start(out=cb, in_=cpb_d)
        # v-cols first (vt runs first), then the two qk column blocks the
        # first pair needs (ot0/ot4: waiting on the FULL wqk slice left the
        # PE sparse right after vt and re-engaged the HAM throttle), then
        # the rest, interleaved with x1
        nc.sync.dma_start(out=wp[:, :, :, 2 * C:3 * C],
                          in_=wpack_d[:, :, :, 2 * C:3 * C])
        nc.sync.dma_start(out=wp[:, :, :, 0:128], in_=wpack_d[:, :, :, 0:128])
        nc.sync.dma_start(out=wp[:, :, :, 512:640], in_=wpack_d[:, :, :, 512:640])
        dma_x2(1, 0)
        nc.sync.dma_start(out=wp[:, :, :, 128:512], in_=wpack_d[:, :, :, 128:512])
        nc.sync.dma_start(out=wp[:, :, :, 640:2 * C], in_=wpack_d[:, :, :, 640:2 * C])
        dma_x2(1, 1)
        nc.sync.dma_start(out=wp[:, :, :, WP_OFF:], in_=wpack_d[:, :, :, WP_OFF:])

        magic = const.tile([NUM_GROUPS, 1], U32)
        nc.gpsimd.memset(magic, 0x5F3759DF)
        ones33 = const.tile([33, HD], BF16)
        nc.gpsimd.memset(ones33, 1.0)

        # ---- HAM warmup: keep the PE active (and the clock un-gated)
        # through the DMA/groupnorm head so vt/qk start at 2.4GHz ----
        wu = None
        if USE_WARM:
            wu = const.tile([128, 512], BF16)
            nc.gpsimd.memset(wu, 0.0)

        def warmup(n):
            if wu is None:
                return
            for i in range(n):
                t = pslot("wup")
                nc.tensor.matmul(t[:, 0:512], lhsT=wu[:, 0:128],
                                 rhs=wu[:, 0:512], start=True, stop=True)

        warmup(N_WARM0 if USE_WARM else 0)

        def w8(p, c0, c1):
            return wp[:, p, :, c0:c1]

        state = [dict() for _ in range(BPC)]

        def emit_gn(b, split=False, warm_mid=0):
            emit_gn_stats(b)
            # the stats psum matmul FIFO-blocks the PE while waiting on the
            # DVE moment chain; warmups emitted here fill the window between
            # it and the (also-blocking) apply matmuls so HAM stays warm
            warmup(warm_mid)
            emit_gn_apply(b, split=split)

        def emit_gn_stats_a(b, cts):
            """bn stats + per-ct channel moments (DVE)."""
            s = state[b]
            cm2s = s.setdefault("cm2s", [None] * CT)
            for ct in cts:
                stats = smalls.tile([128, 2, 6], F32, tag="bnst", name=f"bnst_{b}_{ct}")
                for sg in range(2):
                    nc.vector.bn_stats(out=stats[:, sg, :],
                                       in_=x_sb[b][:, ct, sg * 512:(sg + 1) * 512])
                cmv = smalls.tile([128, 2], F32, tag="cmv", name=f"cmv_{b}_{ct}")
                nc.vector.bn_aggr(out=cmv, in_=stats)
                cm2 = smalls.tile([128, 2], BF16, tag="cm2", name=f"cm2_{b}_{ct}")
                nc.vector.tensor_copy(out=cm2[:, 0:1], in_=cmv[:, 0:1])
                nc.vector.tensor_tensor(out=cm2[:, 1:2], in0=cmv[:, 0:1], in1=cmv[:, 0:1], op=ALU.mult)
                nc.vector.tensor_tensor(out=cm2[:, 1:2], in0=cm2[:, 1:2], in1=cmv[:, 1:2], op=ALU.add)
                cm2s[ct] = cm2

        def emit_gn_stats(b):
            """group-norm stats of x_sb[b]: bn stats + group reduce + quake
            rstd."""
            s = state[b]
            todo = [ct for ct in range(CT)
                    if s.get("cm2s") is None or s["cm2s"][ct] is None]
            if todo:
                emit_gn_stats_a(b, todo)
            cm2s = s["cm2s"]
            ps_g = pslot(f"psg_{b}")
            for ct in range(CT):
                nc.tensor.matmul(ps_g[0:NUM_GROUPS, 0:2],
                                 lhsT=cb[:, CB_GM + ct * 32:CB_GM + (ct + 1) * 32],
                                 rhs=cm2s[ct], start=(ct == 0), stop=(ct == CT - 1))
            gstat = smalls.tile([NUM_GROUPS, 2], BF16, tag="gstat", name=f"gstat_{b}")
            nc.vector.tensor_scalar_mul(out=gstat, in0=ps_g[0:NUM_GROUPS, 0:2], scalar1=1.0 / GS)
            var_g = smalls.tile([NUM_GROUPS, 1], F32, tag="varg", name=f"varg_{b}")
            nc.vector.tensor_tensor(out=var_g, in0=gstat[:, 0:1], in1=gstat[:, 0:1], op=ALU.mult)
            nc.vector.tensor_tensor(out=var_g, in0=gstat[:, 1:2], in1=var_g, op=ALU.subtract)
            nc.vector.tensor_scalar_add(out=var_g, in0=var_g, scalar1=EPS)
            y_n = smalls.tile([NUM_GROUPS, 1], F32, tag="yn", name=f"yn_{b}")
            t_n = smalls.tile([NUM_GROUPS, 1], F32, tag="tn", name=f"tn_{b}")
            nc.vector.tensor_scalar(out=y_n.bitcast(U32), in0=var_g.bitcast(U32),
                                    scalar1=1, scalar2=None, op0=ALU.logical_shift_right)
            nc.vector.tensor_tensor(out=y_n.bitcast(U32), in0=magic,
                                    in1=y_n.bitcast(U32), op=ALU.subtract)
            for _ in range(2):
                nc.vector.tensor_tensor(out=t_n, in0=y_n, in1=y_n, op=ALU.mult)
                nc.vector.scalar_tensor_tensor(out=t_n, in0=t_n, scalar=-0.5,
                                               in1=var_g, op0=ALU.mult, op1=ALU.mult)
                nc.vector.scalar_tensor_tensor(out=y_n, in0=t_n, scalar=1.5,
                                               in1=y_n, op0=ALU.add, op1=ALU.mult)
            nc.vector.tensor_copy(out=gstat[:, 1:2], in_=y_n)
            s["gstat"] = gstat

        def emit_gn_apply(b, split=False):
            s = state[b]
            gstat = s["gstat"]
            xns = [xnpool.tile([128, 2, HW], F8E4, tag="xn", name=f"xn_{b}_{p}")
                   for p in range(NP)]
            for ct in range(CT):
                ps_e = pslot(f"pse_{b}_{ct}")
                nc.tensor.matmul(ps_e[:, 0:2],
                                 lhsT=cb[0:32, CB_EM + ct * 128:CB_EM + (ct + 1) * 128],
                                 rhs=gstat, start=True, stop=True)
                sc = smalls.tile([128, 1], F32, tag="sc", name=f"sc_{b}_{ct}")
                bi = smalls.tile([128, 1], F32, tag="bi", name=f"bi_{b}_{ct}")
                nc.vector.tensor_tensor(out=sc, in0=cp[:, CP_GNG + ct:CP_GNG + ct + 1],
                                        in1=ps_e[:, 1:2], op=ALU.mult)
                nc.vector.tensor_tensor(out=bi, in0=ps_e[:, 0:1], in1=sc, op=ALU.mult)
                nc.vector.tensor_tensor(out=bi, in0=cp[:, CP_GNB + ct:CP_GNB + ct + 1],
                                        in1=bi, op=ALU.subtract)
                # xn is on the critical path for vt/qk: split DVE/GpSimd
                if split:
                    xeng = nc.gpsimd if (ct == 3 and USE_GP) else nc.vector
                else:
                    xeng = nc.gpsimd if (ct >= 2 and USE_GP) else nc.vector
                xeng.tensor_scalar(out=xns[ct // 2][:, ct % 2, :],
                                   in0=x_sb[b][:, ct, :],
                                   scalar1=sc, scalar2=bi, op0=ALU.mult, op1=ALU.add)
            s["xns"] = xns

        def emit_vt(b, sts):
            """vT[s, o] = sum_c xn[c, s] * WvT[c, o] (fp8 DR over kt pairs);
            + 1/16 col at 64 for the scaled rowsum."""
            s = state[b]
            for st in sts:
                ps_v = pslot(f"psv_{b}_{st}")
                for p in range(NP):
                    nc.tensor.matmul(ps_v[:, 0:C],
                                     lhsT=s["xns"][p][:, :, st * 128:(st + 1) * 128],
                                     rhs=w8(p, 2 * C, 3 * C),
                                     start=(p == 0), stop=(p == NP - 1),
                                     perf_mode=DR)
                pv = ps_v[:, 0:C].rearrange("p (h d) -> p h d", h=NUM_HEADS)
                vtaps = s.setdefault("vtaps", [None] * (ST // 2))
                sp, k = st // 2, st % 2
                if vtaps[sp] is None:
                    # head stride 66 (528B) keeps the DoubleRow dual-
                    # subtile step 16B-aligned
                    vtaps[sp] = vtapool.tile([128, 2, NUM_HEADS, HD + 2], F8E4,
                                             tag="vta", name=f"vta_{b}_{sp}")
                    nc.gpsimd.memset(vtaps[sp][:, :, :, HD:HD + 1], ONES_SC)
                dst = vtaps[sp][:, k, :, 0:HD]
                if v_bias:
                    nc.vector.tensor_tensor(
                        out=dst, in0=pv,
                        in1=cp[:, CP_VB:CP_VB + C].rearrange("p (h d) -> p h d", h=NUM_HEADS),
                        op=ALU.add)
                else:
                    nc.vector.tensor_copy(out=dst, in_=pv)

        def emit_qk(b, ots):
            """q/k channel-major (fp8 DR); q drains on ScalarE, padded k
            halves on DVE."""
            s = state[b]
            qks = s.setdefault("qks", [None] * QKT)
            for ot in ots:
                ps_q = pslot(f"psq_{b}_{ot}")
                for p in range(NP):
                    for nh in range(NH):
                        nc.tensor.matmul(ps_q[:, nh * 512:(nh + 1) * 512],
                                         lhsT=w8(p, ot * 128, (ot + 1) * 128),
                                         rhs=s["xns"][p][:, :, nh * 512:(nh + 1) * 512],
                                         start=(p == 0), stop=(p == NP - 1),
                                         perf_mode=DR)
                if ot >= CT:
                    kps = s.setdefault("kps", {})
                    pads = []
                    for j in range(2):
                        # head j's k rows stay at partitions 64j:64j+64 (same
                        # rows its q occupies in the rhs); other half is zero.
                        # tag-stable slot: the zero half written for b=0
                        # persists physically for b=1's reuse.
                        kp = kppool.tile([128, HW], BF16, tag=f"kp{(ot - CT) * 2 + j}",
                                         name=f"kp_{b}_{ot}_{j}")
                        lo, hi = 64 * j, 64 * j + 64
                        if b == 0:
                            nc.gpsimd.memset(kp[0:64, :] if j else kp[64:128, :], 0.0)
                        dst = kp[lo:hi, :]
                        if q_bias:
                            nc.vector.tensor_scalar_add(
                                out=dst, in0=ps_q[lo:hi, :],
                                scalar1=cp[:, CP_QKVB + ot:CP_QKVB + ot + 1][lo:hi])
                        elif b == 0:
                            nc.scalar.activation(out=dst, in_=ps_q[lo:hi, :],
                                                 func=ACTF.Copy)
                        else:
                            nc.vector.tensor_copy(out=dst, in_=ps_q[lo:hi, :])
                        pads.append(kp)
                    kps[ot - CT] = pads
                    continue
                qt = qkvpool.tile([128, HW], BF16, tag="qkv", name=f"qk_{b}_{ot}")
                if q_bias:
                    nc.vector.tensor_scalar_add(out=qt, in0=ps_q,
                                                scalar1=cp[:, CP_QKVB + ot:CP_QKVB + ot + 1])
                else:
                    nc.scalar.activation(out=qt, in_=ps_q, func=ACTF.Copy)
                qks[ot] = qt

        def emit_pair(b, hp, last=False):
            """Head pair (2hp, 2hp+1) as two sequential j-phases sharing one
            AV accumulator.  Full-width S slots (3-deep rotation) -> exps are
            single [128,1024] ops split ACT (table exp) / DVE (Schraudolph)."""
            s = state[b]
            if "hts" not in s:
                s["hts"] = [hpool.tile([128, 2, HW], F8E4, tag="hm",
                                       name=f"hm_{b}_{p}") for p in range(NP)]
            qt2 = s["qks"][hp]
            kpads = s["kps"][hp]
            vtaps = s["vtaps"]
            p, kk = hp // 2, hp % 2
            hus = []
            r2 = rsm.tile([33, HW], F32, tag="r2", name=f"r2_{b}_{hp}")
            ri2 = rsm.tile([33, HW], F32, tag="ri2", name=f"ri2_{b}_{hp}")
            ri2b = None
            if last and not SAFE_LAST:
                ri2b = rsm.tile([33, HW], BF16, tag="ri2b", name=f"ri2b_{b}_{hp}")

            all_slots = [{}, {}]
            all_pexps = [{}, {}]
            ps_os = [None, None]

            def s_mm(j, st):
                t = pslot(f"pss_{b}_{hp}_{j}_{st}")
                # K=128 with zero rows in the k operand: rows of q
                # belonging to the other head hit zero weights
                for nh in range(NH):
                    nc.tensor.matmul(t[:, nh * 512:(nh + 1) * 512],
                                     lhsT=kpads[j][:, st * 128:(st + 1) * 128],
                                     rhs=qt2[:, nh * 512:(nh + 1) * 512],
                                     start=True, stop=True)
                all_slots[j][st] = t

            def emit_exp(j, st):
                sp, k = st // 2, st % 2
                pexps = all_pexps[j]
                if k == 0:
                    pexps[sp] = exppool.tile([128, 2, HW], F8E5, tag="pexp",
                                             name=f"pexp_{b}_{hp}_{j}_{sp}")
                dst = pexps[sp][:, k, :]
                if st not in ACT_TILES[j]:
                    nc.vector.tensor_scalar(out=dst.bitcast(I8),
                                            in0=all_slots[j][st],
                                            scalar1=SCHR8_A, scalar2=SCHR8_B,
                                            op0=ALU.mult, op1=ALU.add)
                else:
                    nc.scalar.activation(out=dst, in_=all_slots[j][st],
                                         func=ACTF.Exp, scale=1.0 / np.sqrt(HD))

            def av_block(j):
                # ALL AV matmuls after the S stream: an AV matmul waiting on
                # an exp would head-of-line block the S matmuls queued
                # behind it (PE FIFO), serializing the phase to the exp chain
                h = 2 * hp + j
                for sp in range(ST // 2):
                    for nh in range(NH):
                        nc.tensor.matmul(
                            ps_os[j][0:HD + 1, nh * 512:(nh + 1) * 512],
                            lhsT=vtaps[sp][:, :, h, 0:HD + 1],
                            rhs=all_pexps[j][sp][:, :, nh * 512:(nh + 1) * 512],
                            start=(sp == 0), stop=(sp == ST // 2 - 1),
                            perf_mode=DR)

            def drain(j):
                # frees the single AV accumulator for the next phase; r rows
                # bounce via SBUF (reciprocal_approx_fast cannot read PSUM)
                ps_o = ps_os[j]
                hu = hupool.tile([HD, HW], BF16, tag="hu", name=f"hu_{b}_{hp}_{j}")
                nc.scalar.activation(out=hu, in_=ps_o[0:HD, :], func=ACTF.Copy)
                hus.append(hu)
                nc.scalar.activation(out=r2[32 * j:32 * j + 1, :],
                                     in_=ps_o[HD:HD + 1, :], func=ACTF.Copy)
                if not (last and not SAFE_LAST):
                    return
                nc.vector.reciprocal_approx_fast(
                    out=ri2[32 * j:32 * j + 1, :],
                    in_=r2[32 * j:32 * j + 1, :])
                if j == 0:
                    # j0 normalizes via the DMA roundtrip (all SBUF-side)
                    # so the single AV accumulator frees for j1's phase
                    # right after the hu/r copies
                    rs = drams.tile([2, HW], F32, tag="rs", name=f"rs_{b}_{hp}")
                    nc.sync.dma_start(out=rs[0:1, :], in_=ri2[0:1, :])
                    rb = rbpool.tile([HD, 2, HW], F32, tag="rb", name=f"rb_{b}_{hp}")
                    rs_bc = bass.AP(tensor=rs.tensor, offset=rs.offset,
                                    ap=[[0, HD]] + [list(rs.ap)[-1]])
                    nc.sync.dma_start(out=rb[:, 0, :], in_=rs_bc)
                    eng0 = nc.gpsimd if USE_GP else nc.vector
                    eng0.tensor_tensor(out=s["hts"][p][0:64, kk, :],
                                       in0=hu, in1=rb[:, 0, :], op=ALU.mult)
                else:
                    # j1 (the true tail): K=1 bf16 broadcast matmul into
                    # rows 64:128 of the drained AV psum + DVE multiply
                    nc.vector.tensor_copy(out=ri2b[32:33, :], in_=ri2[32:33, :])
                    for nh in range(NH):
                        nc.tensor.matmul(ps_os[1][HD:128, nh * 512:(nh + 1) * 512],
                                         lhsT=ones33[32:33, :],
                                         rhs=ri2b[32:33, nh * 512:(nh + 1) * 512],
                                         start=True, stop=True)
                    nc.vector.tensor_tensor(out=s["hts"][p][64:128, kk, :],
                                            in0=hu, in1=ps_os[1][HD:128, :],
                                            op=ALU.mult)

            # phase j0
            ps_os[0] = ps_av.tile([128, HW], F32, tag="psav", name=f"pso_{b}_{hp}_0")
            s_mm(0, 0)
            for st in range(ST):
                emit_exp(0, st)
                if st + 1 < ST:
                    s_mm(0, st + 1)
            # prefill j1's S pipeline before the AV-j0 block so the PE keeps
            # streaming while AV waits on j0's exp tail; j0's drain copies go
            # after j1's first exps so they don't head-of-line block the
            # ACT/DVE queues
            ps_os[1] = ps_av.tile([128, HW], F32, tag="psav", name=f"pso_{b}_{hp}_1")
            s_mm(1, 0)
            s_mm(1, 1)
            av_block(0)
            emit_exp(1, 0)
            s_mm(1, 2)
            emit_exp(1, 1)
            s_mm(1, 3)
            drain(0)
            for st in range(2, ST):
                emit_exp(1, st)
                if st + 2 < ST:
                    s_mm(1, st + 2)
            av_block(1)
            drain(1)

            if last and not SAFE_LAST:
                return None
            nc.vector.reciprocal_approx_fast(out=ri2, in_=r2)
            rs = drams.tile([2, HW], F32, tag="rs", name=f"rs_{b}_{hp}")
            for j in range(2):
                nc.sync.dma_start(out=rs[j:j + 1, :], in_=ri2[32 * j:32 * j + 1, :])
            rb = rbpool.tile([HD, 2, HW], F32, tag="rb", name=f"rb_{b}_{hp}")
            rs_bc = bass.AP(tensor=rs.tensor, offset=rs.offset,
                            ap=[[0, HD]] + list(rs.ap))
            nc.sync.dma_start(out=rb, in_=rs_bc)

            eng = nc.gpsimd if USE_GP else nc.vector

            def finish():
                for j in range(2):
                    eng.tensor_tensor(out=s["hts"][p][j * 64:j * 64 + 64, kk, :],
                                      in0=hus[j], in1=rb[:, j, :], op=ALU.mult)
            return finish

        def emit_proj(b, ots):
            s = state[b]
            for ot in ots:
                yt = ypool.tile([128, HW], BF16, tag="yt", name=f"yt_{b}_{ot}")
                ps_p = pslot(f"psp_{b}_{ot}")
                for p in range(NP):
                    for nh in range(NH):
                        nc.tensor.matmul(ps_p[:, nh * 512:(nh + 1) * 512],
                                         lhsT=w8(p, WP_OFF + ot * 128, WP_OFF + (ot + 1) * 128),
                                         rhs=s["hts"][p][:, :, nh * 512:(nh + 1) * 512],
                                         start=(p == 0), stop=(p == NP - 1),
                                         perf_mode=DR)
                xin = x_sb[b][:, ot, :]
                if p_bias:
                    nc.vector.tensor_scalar(out=yt, in0=ps_p, scalar1=PROJ_SC,
                                            scalar2=cp[:, CP_PROJB + ot:CP_PROJB + ot + 1],
                                            op0=ALU.mult, op1=ALU.add)
                    nc.vector.tensor_tensor(out=yt, in0=yt, in1=xin, op=ALU.add)
                elif b == 1 and ot % 2 == 1:
                    # tail drains: ACT scaled-copy + GpSimd residual add
                    # (ACT/GpSimd are idle at the tail; the 4 serial DVE
                    # drains were the last ~5us of the kernel)
                    yh = ypool.tile([128, HW], BF16, tag="yh", name=f"yh_{b}_{ot}")
                    nc.scalar.activation(out=yh, in_=ps_p, func=ACTF.Copy,
                                         scale=PROJ_SC)
                    nc.gpsimd.tensor_tensor(out=yt, in0=yh, in1=xin, op=ALU.add)
                else:
                    nc.vector.scalar_tensor_tensor(out=yt, in0=ps_p,
                                                   scalar=PROJ_SC, in1=xin,
                                                   op0=ALU.mult, op1=ALU.add)
                nc.sync.dma_start(out=out_d[b, ot * 128:(ot + 1) * 128, :], in_=yt)

        # ---- schedule ----
        emit_gn(0, split=True, warm_mid=8)
        warmup(N_WARM1)
        emit_vt(0, range(ST))
        emit_qk(0, [0, 4])
        f00 = emit_pair(0, 0)
        emit_qk(0, [1, 5])
        emit_gn_stats_a(1, [0, 1])
        warmup(N_WARMB)
        f01 = emit_pair(0, 1)
        emit_qk(0, [2, 6])
        emit_gn_stats(1)
        emit_gn_apply(1)
        f00()
        warmup(N_WARMB)
        f02 = emit_pair(0, 2)
        emit_qk(0, [3, 7])
        emit_vt(1, range(0, 4))
        f01()
        warmup(N_WARMB)
        f03 = emit_pair(0, 3)
        emit_vt(1, range(4, ST))
        emit_qk(1, [0, 4])
        f02()
        warmup(N_WARMB)
        f10 = emit_pair(1, 0)
        emit_qk(1, [1, 5, 2, 6])
        f03()
        emit_proj(0, [0, 1])
        warmup(N_WARMB)
        f11 = emit_pair(1, 1)
        f10()
        emit_qk(1, [3, 7])
        warmup(N_WARMB)
        f12 = emit_pair(1, 2)
        f11()
        emit_proj(0, [2, 3])
        f12()
        warmup(N_WARMB)
        f13 = emit_pair(1, 3, last=True)
        if f13 is not None:
            f13()
        # keep the PE active through the last pair's serial normalize chain:
        # a rethrottle here made ALL 16 proj matmuls run at 1.2GHz
        warmup(N_WARMT)
        emit_proj(1, range(CT))


def make_host_inputs(x, gn_gamma, gn_beta, qkv_w, qkv_b, proj_w, proj_b):
    """Full inputs -> list of per-core in_maps (packed weight/const tensors)."""
    x = np.asarray(x, dtype=np.float32).reshape(B, C, HW).astype(ml_dtypes.bfloat16)
    wqkvT = np.asarray(qkv_w, dtype=np.float32).T          # [C, 3C]
    wprojT = np.asarray(proj_w, dtype=np.float32).T        # [C, C]
    wall = np.concatenate([wqkvT, wprojT], axis=1)         # [C, 4C]
    # TRN e4m3 tops out at 240 (not OCP's 448)
    w8 = np.clip(wall, -240.0, 240.0).astype(ml_dtypes.float8_e4m3fn)
    wpack = np.zeros((128, NP, 2, WPACK_COLS), dtype=ml_dtypes.float8_e4m3fn)
    for p in range(NP):
        for k in range(2):
            kt = 2 * p + k
            wpack[:, p, k, :] = w8[kt * 128:(kt + 1) * 128, :]

    qkv_b = np.asarray(qkv_b, dtype=np.float32)
    v_bias = bool(np.any(qkv_b[2 * C:]))
    cpack = np.zeros((128, CPF_COLS_V if v_bias else CPF_COLS_NOV),
                     dtype=np.float32)
    for ot in range(QKT):
        cpack[:, CP_QKVB + ot] = qkv_b[ot * 128:(ot + 1) * 128]
    for t in range(CT):
        cpack[:, CP_PROJB + t] = np.asarray(proj_b, dtype=np.float32)[t * 128:(t + 1) * 128]
        cpack[:, CP_GNG + t] = np.asarray(gn_gamma, dtype=np.float32)[t * 128:(t + 1) * 128]
        cpack[:, CP_GNB + t] = np.asarray(gn_beta, dtype=np.float32)[t * 128:(t + 1) * 128]
    if v_bias:
        cpack[:, CP_VB:CP_VB + C] = qkv_b[2 * C:3 * C][None, :]

    cpb = np.zeros((128, CB_COLS), dtype=ml_dtypes.bfloat16)
    for t in range(CT):
        for k in range(128):
            cpb[k, CB_GM + t * 32 + (t * 128 + k) // GS] = 1.0
            cpb[(t * 128 + k) // GS, CB_EM + t * 128 + k] = 1.0

    shared = {"wpack": wpack, "cpack": cpack, "cpb": cpb}
    return [dict(shared, x=np.ascontiguousarray(x[i * BPC:(i + 1) * BPC]))
            for i in range(NCORES)]


_NC_CACHE = {}


def _get_nc(q_bias=False, v_bias=False, p_bias=False):
    key = (q_bias, v_bias, p_bias)
    if key not in _NC_CACHE:
        _NC_CACHE[key] = build(q_bias=q_bias, v_bias=v_bias, p_bias=p_bias)
    return _NC_CACHE[key]


def kernel(x, gn_gamma, gn_beta, qkv_w, qkv_b, proj_w, proj_b):
    from concourse.bass_utils import run_bass_kernel_spmd
    qkv_b = np.asarray(qkv_b)
    nc = _get_nc(q_bias=bool(np.any(qkv_b[:2 * C])),
                 v_bias=bool(np.any(qkv_b[2 * C:])),
                 p_bias=bool(np.any(np.asarray(proj_b))))
    in_maps = make_host_inputs(x, gn_gamma, gn_beta, qkv_w, qkv_b, proj_w, proj_b)
    res = run_bass_kernel_spmd(nc, in_maps, list(range(NCORES)))
    out = np.concatenate([res.results[i]["out"] for i in range(NCORES)], axis=0)
    return out.reshape(B, C, H, W).astype(np.float32)
